# revision 16
# baseline (speedup 1.0000x reference)
"""Trainium2 Bass kernel for a 2-layer GCN pair (BRIGHT arch) on 8 NeuronCores.

Layout: cores 0-3 process graph 1, cores 4-7 process graph 2 (one SPMD
program; per-core inputs differ). Within a 4-core group each core owns a
contiguous slice of SLICE node rows (node space padded to NP rows).

Per GCN layer: out = P (A+I) P h + b, with P = diag(rsqrt(deg)),
h = x @ W. Self-loops are explicit self-edges. The sparse aggregation is:
gather rows of h~ = P h by edge src (dma_gather, bf16 256B rows), build a
one-hot S chunk [128 edges x 128 dst_local] on DVE (tensor_scalar
is_equal vs an iota tile), and matmul-accumulate into PSUM per 128-row
dst tile. Layer-1 aggregates feature-major (operand swap) so h2 = agg @ W2
needs no input transpose; a PE transpose + P^2 scale produces the
node-major h~2 slice. The h~2 exchange within the 4-core group is split
into chunked AllGathers pipelined against the layer-1 aggregation (the
chunk-major h2full layout is handled by permuting the layer-2 gather
indices host-side). Layer-2 aggregates node-major and feeds the l1-norm /
comb head directly. dma_gather blocks round-robin across the 4 SWDGE
queues so descriptor generation overlaps DMA drain.
"""

import numpy as np
import ml_dtypes

import concourse.bass as bass
import concourse.tile as tile
from concourse import bacc, mybir
from concourse.bass_utils import run_bass_kernel_spmd

F32 = mybir.dt.float32
BF16 = mybir.dt.bfloat16
I16 = mybir.dt.int16

EPS = 1e-12
GC = 64  # gather-block size in chunks (GC*128 indices per dma_gather)
SG = 16  # S-matrix build granularity in chunks
NQ = 4   # SWDGE queues available (queue 0 left to regular DMA traffic)
GATHER_QS = (1, 2, 3)  # queues used for dma_gather round-robin
SINGLE_PACKET = False

_prog_cache: dict = {}


# ---------------------------------------------------------------- host prep

def _chunk_bounds(TLOC):
    """Tile-index boundaries for the chunked h~2 AllGather pipeline."""
    if TLOC < 8:
        mid = (TLOC + 1) // 2
        return [0, mid, TLOC]
    fr = [0.0, 0.26, 0.52, 0.72, 0.88, 1.0]
    b = sorted(set(int(round(f * TLOC)) for f in fr))
    return b


def _perm_from_bounds(bounds, SLICE, NP, group_size):
    """Node id -> row in the chunk-major h2full table."""
    p = np.empty(NP, np.int64)
    for s, e in zip(bounds[:-1], bounds[1:]):
        rs, re = s * 128, e * 128
        n_rows = re - rs
        for g in range(group_size):
            lo = g * SLICE + rs
            p[lo:lo + n_rows] = (group_size * rs + g * n_rows
                                 + np.arange(n_rows))
    return p


def _pack_idx(flat):
    """edge j -> wrapped [16, n/16] then replicated to [128, n/16] int16."""
    n = flat.shape[0]
    assert n % 16 == 0
    w = flat.reshape(n // 16, 16).T.astype(np.int16)  # [16, n/16]
    return np.tile(w, (8, 1))


def _pack_cols(flat, width):
    """edge j -> [width, n/width] column-per-chunk layout."""
    n = flat.shape[0]
    assert n % width == 0
    return np.ascontiguousarray(flat.reshape(n // width, width).T)


def _prep_graph(edge_index, N, NP, SLICE, ABOUND, OVER, n_cores, src_map=None):
    """Per-core edge streams for one graph.

    src_map (if given) remaps src node ids to table rows before the A/B
    bucket classification (used for the permuted layer-2 table layout).
    Returns (dinv, dinvsq, per_core list of per-tile bucket lists).
    """
    src = np.asarray(edge_index[0], dtype=np.int64)
    dst = np.asarray(edge_index[1], dtype=np.int64)
    deg = np.bincount(dst, minlength=N).astype(np.float64) + 1.0
    dinv = (1.0 / np.sqrt(deg)).astype(np.float32)
    dinvsq = (1.0 / deg).astype(np.float32)

    # self-loops handled densely in the tile-close path (not gathered)
    src_a = src_map[src] if src_map is not None else src
    dst_a = dst

    TLOC = SLICE // 128
    cores = []
    for c in range(n_cores):
        lo, hi = c * SLICE, (c + 1) * SLICE
        sel = (dst_a >= lo) & (dst_a < hi)
        s = src_a[sel]
        d = dst_a[sel] - lo
        t_id = d // 128
        dloc = d % 128
        order = np.argsort(t_id, kind="stable")
        s, dloc, t_id = s[order], dloc[order], t_id[order]
        tiles = {}
        for t in range(TLOC):
            m = t_id == t
            st, dt_ = s[m], dloc[m]
            fa = st < OVER
            fb = st >= ABOUND
            fl = ~fa & ~fb
            tiles[t] = (
                (st[fa], dt_[fa]),
                (st[fl], dt_[fl]),
                (st[fb], dt_[fb]),
            )
        cores.append(tiles)
    return dinv, dinvsq, cores


def _split_tile(tile3, KA_t, OVER):
    """Balanced A/B assignment: fill A up to KA_t*128 with flex edges."""
    (sa, da), (sf, df), (sb, db) = tile3
    room = max(KA_t * 128 - len(sa), 0)
    x = min(room, len(sf))
    sA = np.concatenate([sa, sf[:x]])
    dA = np.concatenate([da, df[:x]])
    sB = np.concatenate([sf[x:] - OVER, sb - OVER])
    dB = np.concatenate([df[x:], db])
    return (sA, dA), (sB, dB)


def _slot_counts(graph_cores_list, TLOC):
    """Shared per-tile A/B slot counts (max across all datasets)."""
    KA = np.zeros(TLOC, np.int64)
    KB = np.zeros(TLOC, np.int64)
    for cores in graph_cores_list:
        for tiles in cores:
            for t in range(TLOC):
                (sa, _), _, _ = tiles[t]
                KA[t] = max(KA[t], (len(sa) + 127) // 128)
    KA = np.maximum(KA, 1)
    for cores in graph_cores_list:
        for tiles in cores:
            for t in range(TLOC):
                (sa, _), (sf, _), (sb, _) = tiles[t]
                x = min(max(KA[t] * 128 - len(sa), 0), len(sf))
                nB = len(sf) - x + len(sb)
                KB[t] = max(KB[t], (nB + 127) // 128)
    KB = np.maximum(KB, 1)
    return KA, KB


def _build_streams(tiles, KA, KB, TLOC, OVER):
    """Flatten one core's per-tile bucket lists into padded chunk streams."""
    idxA, dlA, idxB, dlB = [], [], [], []
    for t in range(TLOC):
        (sa, da), (sb, db) = _split_tile(tiles[t], KA[t], OVER)
        for (s_, d_, K, idx_l, dl_l) in (
            (sa, da, KA[t], idxA, dlA),
            (sb, db, KB[t], idxB, dlB),
        ):
            # ascending table rows within the tile bucket for DRAM locality
            o = np.argsort(s_, kind="stable")
            s_, d_ = s_[o], d_[o]
            n = s_.shape[0]
            slots = K * 128
            assert n <= slots, (n, slots)
            si = np.zeros(slots, np.int64)
            di = np.full(slots, -1.0, np.float32)
            si[:n] = s_
            di[:n] = d_.astype(np.float32)
            idx_l.append(si)
            dl_l.append(di)
    idxA = np.concatenate(idxA) if idxA else np.zeros(0, np.int64)
    idxB = np.concatenate(idxB) if idxB else np.zeros(0, np.int64)
    dlA = np.concatenate(dlA) if dlA else np.zeros(0, np.float32)
    dlB = np.concatenate(dlB) if dlB else np.zeros(0, np.float32)
    return (
        _pack_idx(idxA), _pack_cols(dlA, 128),
        _pack_idx(idxB), _pack_cols(dlB, 128),
    )


def _cols_from_vec(v_padded, TL):
    """[TL*128] -> [128, TL] per-tile columns."""
    return np.ascontiguousarray(v_padded.reshape(TL, 128).T)


# ---------------------------------------------------------------- builder

def _build_program(NP, SLICE, ABOUND, OVER, KA, KB, KA2, KB2, bounds,
                   n_cores_total, group_size):
    TLOC = SLICE // 128
    TFULL = NP // 128
    CA, CB = int(sum(KA)), int(sum(KB))
    CA2, CB2 = int(sum(KA2)), int(sum(KB2))
    LA, LB = CA * 128, CB * 128
    LA2, LB2 = CA2 * 128, CB2 * 128

    nc = bacc.Bacc("TRN2", target_bir_lowering=False, debug=False,
                   num_devices=n_cores_total, num_swdge_queues=NQ,
                   dynamic_dma_scratch_size=65536)

    xT = nc.dram_tensor("xT", [128, NP], BF16, kind="ExternalInput")
    rwrT = nc.dram_tensor("rwrT", [128, SLICE], BF16, kind="ExternalInput")
    xTloc = nc.dram_tensor("xTloc", [128, SLICE], BF16, kind="ExternalInput")
    idxA = nc.dram_tensor("idxA", [128, max(LA // 16, 1)], I16, kind="ExternalInput")
    idxB = nc.dram_tensor("idxB", [128, max(LB // 16, 1)], I16, kind="ExternalInput")
    dlA = nc.dram_tensor("dlA", [128, max(CA, 1)], BF16, kind="ExternalInput")
    dlB = nc.dram_tensor("dlB", [128, max(CB, 1)], BF16, kind="ExternalInput")
    idxA2 = nc.dram_tensor("idxA2", [128, max(LA2 // 16, 1)], I16, kind="ExternalInput")
    idxB2 = nc.dram_tensor("idxB2", [128, max(LB2 // 16, 1)], I16, kind="ExternalInput")
    dlA2 = nc.dram_tensor("dlA2", [128, max(CA2, 1)], BF16, kind="ExternalInput")
    dlB2 = nc.dram_tensor("dlB2", [128, max(CB2, 1)], BF16, kind="ExternalInput")
    dinv_full = nc.dram_tensor("dinv_full", [128, TFULL], F32, kind="ExternalInput")
    dinv_loc = nc.dram_tensor("dinv_loc", [128, TLOC], F32, kind="ExternalInput")
    dinvsq_loc = nc.dram_tensor("dinvsq_loc", [128, TLOC], F32, kind="ExternalInput")
    W1 = nc.dram_tensor("W1", [128, 128], BF16, kind="ExternalInput")
    W2 = nc.dram_tensor("W2", [128, 128], BF16, kind="ExternalInput")
    linW = nc.dram_tensor("linW", [128, 128], BF16, kind="ExternalInput")
    combWt = nc.dram_tensor("combWt", [128, 128], BF16, kind="ExternalInput")
    combWb = nc.dram_tensor("combWb", [128, 128], BF16, kind="ExternalInput")
    iota = nc.dram_tensor("iota", [128, 128], BF16, kind="ExternalInput")
    ident = nc.dram_tensor("ident", [128, 128], BF16, kind="ExternalInput")
    emd_out = nc.dram_tensor("emd_out", [SLICE, 128], F32, kind="ExternalOutput")

    groups = [
        list(range(g * group_size, (g + 1) * group_size))
        for g in range(n_cores_total // group_size)
    ]

    with tile.TileContext(nc) as tc:
        with tc.tile_pool(name="dram", bufs=1, space="DRAM") as dram, \
             tc.tile_pool(name="const", bufs=1) as cp, \
             tc.tile_pool(name="xload", bufs=3) as xp, \
             tc.tile_pool(name="blkA", bufs=2) as bap, \
             tc.tile_pool(name="blkB", bufs=2) as bbp, \
             tc.tile_pool(name="sA", bufs=2) as sap, \
             tc.tile_pool(name="sB", bufs=2) as sbp, \
             tc.tile_pool(name="work", bufs=3) as wp, \
             tc.tile_pool(name="norm", bufs=4) as npools, \
             tc.tile_pool(name="ps_agg", bufs=2, space="PSUM") as ps_agg, \
             tc.tile_pool(name="ps_aux", bufs=2, space="PSUM") as ps_aux, \
             tc.tile_pool(name="ps_tr", bufs=2, space="PSUM") as ps_tr:
            # PSUM budget (8 banks): ps_agg tag "agg" x2, ps_aux tag "mm" x2,
            # ps_tr tag "tr" x2 -> 6 bank-padded slots.

            h1t = dram.tile([NP, 128], BF16)
            h2slice = dram.tile([SLICE, 128], BF16)
            h2full = dram.tile([NP, 128], BF16)

            # ---- constants / streams in SBUF
            def cload(t_dram, shape, dt, tag):
                t_sb = cp.tile(shape, dt, tag=tag)
                nc.sync.dma_start(t_sb[:], t_dram[:, :])
                return t_sb

            idxA_t = cload(idxA, [128, max(LA // 16, 1)], I16, "idxA")
            idxB_t = cload(idxB, [128, max(LB // 16, 1)], I16, "idxB")
            dlA_t = cload(dlA, [128, max(CA, 1)], BF16, "dlA")
            dlB_t = cload(dlB, [128, max(CB, 1)], BF16, "dlB")
            idxA2_t = cload(idxA2, [128, max(LA2 // 16, 1)], I16, "idxA2")
            idxB2_t = cload(idxB2, [128, max(LB2 // 16, 1)], I16, "idxB2")
            dlA2_t = cload(dlA2, [128, max(CA2, 1)], BF16, "dlA2")
            dlB2_t = cload(dlB2, [128, max(CB2, 1)], BF16, "dlB2")
            dinvf_t = cload(dinv_full, [128, TFULL], F32, "dinvf")
            dinvl_t = cload(dinv_loc, [128, TLOC], F32, "dinvl")
            dinvsq_t = cload(dinvsq_loc, [128, TLOC], F32, "dinvsq")
            W1_t = cload(W1, [128, 128], BF16, "W1")
            W2_t = cload(W2, [128, 128], BF16, "W2")
            linW_t = cload(linW, [128, 128], BF16, "linW")
            combWt_t = cload(combWt, [128, 128], BF16, "combWt")
            combWb_t = cload(combWb, [128, 128], BF16, "combWb")
            iota_t = cload(iota, [128, 128], BF16, "iota")
            ident_t = cload(ident, [128, 128], BF16, "ident")

            Copy = mybir.ActivationFunctionType.Copy

            def l1norm_scale(src_ap, out_tile_ap):
                """out = src / max(sum|src|, EPS), per-partition rows."""
                s_sum = npools.tile([128, 1], F32, tag="nsum")
                nc.vector.reduce_sum(
                    s_sum[:], src_ap, axis=mybir.AxisListType.X,
                    apply_absolute_value=True)
                s_max = npools.tile([128, 1], F32, tag="nmax")
                nc.vector.tensor_scalar_max(s_max[:], s_sum[:], EPS)
                r = npools.tile([128, 1], F32, tag="nrec")
                nc.vector.reciprocal(r[:], s_max[:])
                nc.scalar.activation(out_tile_ap, src_ap, Copy, scale=r[:, 0:1])

            # ================= stage 1: h~1 full table (redundant per core)
            for tq in range(TFULL // 4):
                x4 = xp.tile([128, 4, 128], BF16, tag="x4")
                nc.sync.dma_start(x4[:], xT[:, tq * 512:(tq + 1) * 512]
                                  .rearrange("p (j f) -> p j f", j=4))
                o4 = xp.tile([128, 4, 128], BF16, tag="o4")
                for j in range(4):
                    t = tq * 4 + j
                    ps = ps_aux.tile([128, 128], F32, tag="mm")
                    nc.tensor.matmul(ps[:], lhsT=x4[:, j, :], rhs=W1_t[:],
                                     start=True, stop=True)
                    nc.scalar.activation(o4[:, j, :], ps[:], Copy,
                                         scale=dinvf_t[:, t:t + 1])
                nc.sync.dma_start(
                    h1t[tq * 512:(tq + 1) * 512, :]
                    .rearrange("(j p) f -> p j f", p=128),
                    o4[:])

            qctr = [0]

            def agg_pass(tableA_ap, tableB_ap, node_major, KAv, KBv,
                         idxA_sb, idxB_sb, dlA_sb, dlB_sb, CAv, CBv):
                """Runs the chunked aggregation over all local tiles.

                node_major=False: psum[dim, dst] += Hg^T-style (lhsT=Hg, rhs=S)
                node_major=True:  psum[dst, dim] += (lhsT=S, rhs=Hg)
                Yields (t, psum_tile) at each tile close.
                """
                qA = qB = 0
                blkA_t = blkB_t = sblkA_t = sblkB_t = None
                for t in range(TLOC):
                    ps = ps_agg.tile([128, 128], F32,
                                     tag="agg")
                    done = 0
                    for (K, stream_q, idx_t, dl_t, table_ap, pool, spool,
                         which) in (
                        (KAv[t], qA, idxA_sb, dlA_sb, tableA_ap, bap, sap, "A"),
                        (KBv[t], qB, idxB_sb, dlB_sb, tableB_ap, bbp, sbp, "B"),
                    ):
                        q = stream_q
                        CTOT = CAv if which == "A" else CBv
                        for i in range(K):
                            if q % GC == 0:
                                cb = min(GC, CTOT - q)
                                blk = pool.tile([128, GC, 128], BF16,
                                                tag="blk" + which)
                                nc.gpsimd.dma_gather(
                                    blk[:, :cb, :], table_ap,
                                    idx_t[:, q * 8:(q + cb) * 8],
                                    num_idxs=cb * 128, num_idxs_reg=cb * 128,
                                    elem_size=128, single_packet=SINGLE_PACKET,
                                    queue_num=GATHER_QS[qctr[0] % len(GATHER_QS)])
                                qctr[0] += 1
                                if which == "A":
                                    blkA_t = blk
                                else:
                                    blkB_t = blk
                            if q % SG == 0:
                                sb_ = min(SG, CTOT - q)
                                sblk = spool.tile([128, SG, 128], BF16,
                                                  tag="sblk" + which)
                                nc.vector.tensor_tensor(
                                    out=sblk[:, :sb_, :],
                                    in0=iota_t[:].unsqueeze(1)
                                        .broadcast_to([128, sb_, 128]),
                                    in1=dl_t[:, q:q + sb_].unsqueeze(2)
                                        .broadcast_to([128, sb_, 128]),
                                    op=mybir.AluOpType.is_equal)
                                if which == "A":
                                    sblkA_t = sblk
                                else:
                                    sblkB_t = sblk
                            blk = blkA_t if which == "A" else blkB_t
                            sblk = sblkA_t if which == "A" else sblkB_t
                            s_t = sblk[:, q % SG, :]
                            hg = blk[:, q % GC, :]
                            if node_major:
                                nc.tensor.matmul(ps[:], lhsT=s_t, rhs=hg,
                                                 start=(done == 0), stop=False)
                            else:
                                nc.tensor.matmul(ps[:], lhsT=hg, rhs=s_t,
                                                 start=(done == 0), stop=False)
                            q += 1
                            done += 1
                        if which == "A":
                            qA = q
                        else:
                            qB = q
                    yield t, ps

            # ================= stage 2: layer-1 agg (feature-major) -> h~2
            # slice, with the group AllGather pipelined chunk-by-chunk
            cc_next = 0  # index into bounds of the next chunk to exchange
            for t, ps in agg_pass(h1t[0:ABOUND, :], h1t[OVER:NP, :], False,
                                  KA, KB, idxA_t, idxB_t, dlA_t, dlB_t,
                                  CA, CB):
                # self-loop term: += W1^T @ (dinv*X_T)[:, own tile]
                xl = wp.tile([128, 128], BF16, tag="xl")
                nc.sync.dma_start(xl[:], xTloc[:, t * 128:(t + 1) * 128])
                nc.tensor.matmul(ps[:], lhsT=W1_t[:], rhs=xl[:],
                                 start=False, stop=True)
                # close: aggT[dim, dst] -> h~2 slice tile (node-major, scaled)
                aggT_sb = wp.tile([128, 128], BF16, tag="aggT")
                nc.scalar.activation(aggT_sb[:], ps[:], Copy)
                h2T_ps = ps_aux.tile([128, 128], F32, tag="mm")
                nc.tensor.matmul(h2T_ps[:], lhsT=W2_t[:], rhs=aggT_sb[:],
                                 start=True, stop=True)
                h2T_sb = wp.tile([128, 128], BF16, tag="h2Ts")
                nc.scalar.activation(h2T_sb[:], h2T_ps[:], Copy)
                h2_ps = ps_tr.tile([128, 128], BF16, tag="tr")
                nc.tensor.transpose(h2_ps[:], h2T_sb[:], ident_t[:])
                h2_sb = wp.tile([128, 128], BF16, tag="h2s")
                nc.scalar.activation(h2_sb[:], h2_ps[:], Copy,
                                     scale=dinvsq_t[:, t:t + 1])
                nc.sync.dma_start(h2slice[t * 128:(t + 1) * 128, :], h2_sb[:])
                # chunk complete -> exchange it while later tiles aggregate
                if t + 1 == bounds[cc_next + 1]:
                    rs, re = bounds[cc_next] * 128, bounds[cc_next + 1] * 128
                    nc.gpsimd.collective_compute(
                        "AllGather", mybir.AluOpType.bypass,
                        replica_groups=groups,
                        ins=[h2slice[rs:re, :].opt()],
                        outs=[h2full[group_size * rs:group_size * re, :].opt()])
                    cc_next += 1

            # ================= stage 4+5: layer-2 agg (node-major) + head
            for t, ps in agg_pass(h2full[0:ABOUND, :], h2full[OVER:NP, :], True,
                                  KA2, KB2, idxA2_t, idxB2_t, dlA2_t, dlB2_t,
                                  CA2, CB2):
                # self-loop term: += h~2[own tile] (identity matmul)
                h2s = wp.tile([128, 128], BF16, tag="h2self")
                nc.sync.dma_start(h2s[:], h2slice[t * 128:(t + 1) * 128, :])
                nc.tensor.matmul(ps[:], lhsT=ident_t[:], rhs=h2s[:],
                                 start=False, stop=True)
                # g = l1norm(dinv * agg2)
                g_pre = wp.tile([128, 128], F32, tag="gpre")
                nc.scalar.activation(g_pre[:], ps[:], Copy,
                                     scale=dinvl_t[:, t:t + 1])
                g_bf = wp.tile([128, 128], BF16, tag="gbf")
                l1norm_scale(g_pre[:], g_bf[:])
                gT_ps = ps_tr.tile([128, 128], BF16, tag="tr")
                nc.tensor.transpose(gT_ps[:], g_bf[:], ident_t[:])
                gT_sb = wp.tile([128, 128], BF16, tag="gT")
                nc.scalar.activation(gT_sb[:], gT_ps[:], Copy)

                # pos = l1norm(rwr @ linW)
                rw = wp.tile([128, 128], BF16, tag="rw")
                nc.sync.dma_start(rw[:], rwrT[:, t * 128:(t + 1) * 128])
                pos_ps = ps_aux.tile([128, 128], F32, tag="mm")
                nc.tensor.matmul(pos_ps[:], lhsT=rw[:], rhs=linW_t[:],
                                 start=True, stop=True)
                pos_bf = wp.tile([128, 128], BF16, tag="posbf")
                l1norm_scale(pos_ps[:], pos_bf[:])
                posT_ps = ps_tr.tile([128, 128], BF16, tag="tr")
                nc.tensor.transpose(posT_ps[:], pos_bf[:], ident_t[:])
                posT_sb = wp.tile([128, 128], BF16, tag="posT")
                nc.scalar.activation(posT_sb[:], posT_ps[:], Copy)

                # emd = l1norm(concat(pos, g) @ combW)
                emd_ps = ps_aux.tile([128, 128], F32, tag="mm")
                nc.tensor.matmul(emd_ps[:], lhsT=posT_sb[:], rhs=combWt_t[:],
                                 start=True, stop=False)
                nc.tensor.matmul(emd_ps[:], lhsT=gT_sb[:], rhs=combWb_t[:],
                                 start=False, stop=True)
                emd_f = wp.tile([128, 128], F32, tag="emdf")
                l1norm_scale(emd_ps[:], emd_f[:])
                nc.sync.dma_start(emd_out[t * 128:(t + 1) * 128, :], emd_f[:])

    nc.compile()
    return nc


# ---------------------------------------------------------------- kernel

def _run(inputs, N, E, n_cores_total=8, group_size=4):
    n_groups = n_cores_total // group_size
    assert n_groups == 2
    SLICE = ((N + group_size * 128 - 1) // (group_size * 128)) * 128
    NP = SLICE * group_size
    ABOUND = min(32768, NP)
    OVER = max(NP - 32768, 0)
    assert NP - OVER <= 32768
    TLOC = SLICE // 128
    TFULL = NP // 128
    assert TFULL % 4 == 0

    bf = ml_dtypes.bfloat16

    bounds = _chunk_bounds(TLOC)
    perm = _perm_from_bounds(bounds, SLICE, NP, group_size)

    graphs = []
    for g in range(2):
        ei = inputs["edge_index1" if g == 0 else "edge_index2"]
        dinv, dinvsq, cores = _prep_graph(ei, N, NP, SLICE, ABOUND, OVER,
                                          group_size)
        _, _, cores2 = _prep_graph(ei, N, NP, SLICE, ABOUND, OVER,
                                   group_size, src_map=perm)
        graphs.append((dinv, dinvsq, cores, cores2))

    KA, KB = _slot_counts([g[2] for g in graphs], TLOC)
    KA2, KB2 = _slot_counts([g[3] for g in graphs], TLOC)

    key = (NP, SLICE, ABOUND, OVER, tuple(KA), tuple(KB),
           tuple(KA2), tuple(KB2), tuple(bounds), n_cores_total, group_size)
    if key not in _prog_cache:
        _prog_cache[key] = _build_program(
            NP, SLICE, ABOUND, OVER, KA, KB, KA2, KB2, bounds,
            n_cores_total, group_size)
    nc = _prog_cache[key]

    iota_np = np.broadcast_to(
        np.arange(128, dtype=np.float32), (128, 128)).astype(bf)
    ident_np = np.eye(128, dtype=np.float32).astype(bf)
    W1_np = np.asarray(inputs["conv1_W"], np.float32).astype(bf)
    W2_np = np.asarray(inputs["conv2_W"], np.float32).astype(bf)
    linW_np = np.asarray(inputs["lin_W"], np.float32).astype(bf)
    combW = np.asarray(inputs["comb_W"], np.float32)
    combWt_np = combW[:128].astype(bf)
    combWb_np = combW[128:].astype(bf)

    in_maps = []
    for core in range(n_cores_total):
        g = core // group_size
        c = core % group_size
        dinv, dinvsq, cores, cores2 = graphs[g]
        x = np.asarray(inputs["x1" if g == 0 else "x2"], np.float32)
        rwr = np.asarray(inputs["rwr1_emd" if g == 0 else "rwr2_emd"],
                         np.float32)

        dinv_p = np.ones(NP, np.float32)
        dinv_p[:N] = dinv
        dinvsq_p = np.ones(NP, np.float32)
        dinvsq_p[:N] = dinvsq

        xT = np.zeros((128, NP), np.float32)
        xT[:, :N] = x.T
        rwrT = np.zeros((128, SLICE), np.float32)
        lo, hi = c * SLICE, min((c + 1) * SLICE, N)
        if hi > lo:
            rwrT[:, :hi - lo] = rwr[lo:hi].T

        iA, dA, iB, dB = _build_streams(cores[c], KA, KB, TLOC, OVER)
        iA2, dA2, iB2, dB2 = _build_streams(cores2[c], KA2, KB2, TLOC, OVER)

        sl = slice(c * SLICE, (c + 1) * SLICE)
        xTloc = xT[:, sl] * dinv_p[sl][None, :]
        in_maps.append({
            "xT": xT.astype(bf),
            "rwrT": rwrT.astype(bf),
            "xTloc": xTloc.astype(bf),
            "idxA": iA, "idxB": iB,
            "dlA": dA.astype(bf), "dlB": dB.astype(bf),
            "idxA2": iA2, "idxB2": iB2,
            "dlA2": dA2.astype(bf), "dlB2": dB2.astype(bf),
            "dinv_full": _cols_from_vec(dinv_p, TFULL),
            "dinv_loc": _cols_from_vec(dinv_p[sl], TLOC),
            "dinvsq_loc": _cols_from_vec(dinvsq_p[sl], TLOC),
            "W1": W1_np, "W2": W2_np, "linW": linW_np,
            "combWt": combWt_np, "combWb": combWb_np,
            "iota": iota_np, "ident": ident_np,
        })

    import os
    if os.environ.get("GCN_SIM"):
        from concourse.bass_interp import MultiCoreSim
        sim = MultiCoreSim(nc, num_cores=n_cores_total, trace=False,
                           require_finite=False, require_nnan=False)
        cores = list(sim.cores.values())
        for c, core_sim in enumerate(cores):
            for k, v in in_maps[c].items():
                core_sim.tensor(k)[:] = v
        sim.simulate(check_with_hw=False)

        class _R:
            results = [{"emd_out": np.array(core_sim.tensor("emd_out"))}
                       for core_sim in cores]
        res = _R()
    else:
        trace = bool(os.environ.get("GCN_TRACE"))
        if trace:
            import sys, types
            if "antenv.axon_hooks" not in sys.modules:
                mod = types.ModuleType("antenv.axon_hooks")
                mod._hook = None
                mod.set_axon_ntff_profile_hook = \
                    lambda h: setattr(mod, "_hook", h)
                mod.get_axon_ntff_profile_hook = lambda: mod._hook
                sys.modules["antenv.axon_hooks"] = mod
                from trn_agent_boot.trn_boot import _ntff_profile_via_ctypes
                mod.set_axon_ntff_profile_hook(
                    _ntff_profile_via_ctypes('/opt/axon/libaxon_pjrt.so'))
        res = run_bass_kernel_spmd(nc, in_maps,
                                   core_ids=list(range(n_cores_total)),
                                   trace=trace)
        if trace:
            print(f"HW exec time: {res.exec_time_ns} ns "
                  f"(mean {res.mean_exec_time_ns}, "
                  f"core {res.max_exec_time_core_id})")
            if res.instructions_and_trace:
                print("trace:", res.instructions_and_trace[1])

    outs = []
    for g in range(2):
        parts = [res.results[g * group_size + c]["emd_out"]
                 for c in range(group_size)]
        outs.append(np.concatenate(parts, axis=0)[:N])
    return outs[0], outs[1]


def kernel(rwr1_emd, rwr2_emd, x1, x2, edge_index1, edge_index2,
           lin_W, lin_b, conv1_W, conv1_b, conv2_W, conv2_b,
           comb_W, comb_b):
    for name, b in (("lin_b", lin_b), ("conv1_b", conv1_b),
                    ("conv2_b", conv2_b), ("comb_b", comb_b)):
        if np.any(np.asarray(b) != 0):
            raise NotImplementedError(f"nonzero bias {name} not supported")
    inputs = dict(rwr1_emd=rwr1_emd, rwr2_emd=rwr2_emd, x1=x1, x2=x2,
                  edge_index1=edge_index1, edge_index2=edge_index2,
                  lin_W=lin_W, conv1_W=conv1_W, conv2_W=conv2_W,
                  comb_W=comb_W)
    N = np.asarray(x1).shape[0]
    E = np.asarray(edge_index1).shape[1]
    return _run(inputs, N, E)


# revision 18
# speedup vs baseline: 1.1609x; 1.1609x over previous
"""Trainium2 Bass kernel for a 2-layer GCN pair (BRIGHT arch) on 8 NeuronCores.

Layout: cores 0-3 process graph 1, cores 4-7 process graph 2 (one SPMD
program; per-core inputs differ). Within a 4-core group each core owns a
contiguous slice of SLICE node rows (node space padded to NP rows).

Per GCN layer: out = P (A+I) P h + b, with P = diag(rsqrt(deg)),
h = x @ W. Self-loops are explicit self-edges. The sparse aggregation is:
gather rows of h~ = P h by edge src (dma_gather, bf16 256B rows), build a
one-hot S chunk [128 edges x 128 dst_local] on DVE (tensor_scalar
is_equal vs an iota tile), and matmul-accumulate into PSUM per 128-row
dst tile. Layer-1 aggregates feature-major (operand swap) so h2 = agg @ W2
needs no input transpose; a PE transpose + P^2 scale produces the
node-major h~2 slice. The h~2 exchange within the 4-core group is split
into chunked AllGathers pipelined against the layer-1 aggregation (the
chunk-major h2full layout is handled by permuting the layer-2 gather
indices host-side). Layer-2 aggregates node-major and feeds the l1-norm /
comb head directly. dma_gather blocks round-robin across the 4 SWDGE
queues so descriptor generation overlaps DMA drain.
"""

import numpy as np
import ml_dtypes

import concourse.bass as bass
import concourse.tile as tile
from concourse import bacc, mybir
from concourse.bass_utils import run_bass_kernel_spmd

F32 = mybir.dt.float32
BF16 = mybir.dt.bfloat16
I16 = mybir.dt.int16

EPS = 1e-12
GC = 32  # gather-block size in chunks (GC*128 indices per dma_gather)
SG = 32  # S-matrix build granularity in chunks
NQ = 4   # SWDGE queues available (queue 0 left to regular DMA traffic)
GATHER_QS = (1, 2, 3)  # queues used for dma_gather round-robin
SINGLE_PACKET = False

_prog_cache: dict = {}


# ---------------------------------------------------------------- host prep

def _chunk_bounds(TLOC):
    """Tile-index boundaries for the chunked h~2 AllGather pipeline."""
    if TLOC < 8:
        mid = (TLOC + 1) // 2
        return [0, mid, TLOC]
    fr = [0.0, 0.26, 0.52, 0.72, 0.88, 0.96, 1.0]
    b = sorted(set(int(round(f * TLOC)) for f in fr))
    return b


def _perm_from_bounds(bounds, SLICE, NP, group_size):
    """Node id -> row in the chunk-major h2full table."""
    p = np.empty(NP, np.int64)
    for s, e in zip(bounds[:-1], bounds[1:]):
        rs, re = s * 128, e * 128
        n_rows = re - rs
        for g in range(group_size):
            lo = g * SLICE + rs
            p[lo:lo + n_rows] = (group_size * rs + g * n_rows
                                 + np.arange(n_rows))
    return p


def _pack_idx(flat):
    """edge j -> wrapped [16, n/16] then replicated to [128, n/16] int16."""
    n = flat.shape[0]
    assert n % 16 == 0
    w = flat.reshape(n // 16, 16).T.astype(np.int16)  # [16, n/16]
    return np.tile(w, (8, 1))


def _pack_cols(flat, width):
    """edge j -> [width, n/width] column-per-chunk layout."""
    n = flat.shape[0]
    assert n % width == 0
    return np.ascontiguousarray(flat.reshape(n // width, width).T)


def _prep_graph(edge_index, N, NP, SLICE, ABOUND, OVER, n_cores, src_map=None):
    """Per-core edge streams for one graph.

    src_map (if given) remaps src node ids to table rows before the A/B
    bucket classification (used for the permuted layer-2 table layout).
    Returns (dinv, dinvsq, per_core list of per-tile bucket lists).
    """
    src = np.asarray(edge_index[0], dtype=np.int64)
    dst = np.asarray(edge_index[1], dtype=np.int64)
    deg = np.bincount(dst, minlength=N).astype(np.float64) + 1.0
    dinv = (1.0 / np.sqrt(deg)).astype(np.float32)
    dinvsq = (1.0 / deg).astype(np.float32)

    # self-loops handled densely in the tile-close path (not gathered)
    src_a = src_map[src] if src_map is not None else src
    dst_a = dst

    TLOC = SLICE // 128
    cores = []
    for c in range(n_cores):
        lo, hi = c * SLICE, (c + 1) * SLICE
        sel = (dst_a >= lo) & (dst_a < hi)
        s = src_a[sel]
        d = dst_a[sel] - lo
        t_id = d // 128
        dloc = d % 128
        order = np.argsort(t_id, kind="stable")
        s, dloc, t_id = s[order], dloc[order], t_id[order]
        tiles = {}
        for t in range(TLOC):
            m = t_id == t
            st, dt_ = s[m], dloc[m]
            fa = st < OVER
            fb = st >= ABOUND
            fl = ~fa & ~fb
            tiles[t] = (
                (st[fa], dt_[fa]),
                (st[fl], dt_[fl]),
                (st[fb], dt_[fb]),
            )
        cores.append(tiles)
    return dinv, dinvsq, cores


def _split_tile(tile3, KA_t, OVER):
    """Balanced A/B assignment: fill A up to KA_t*128 with flex edges."""
    (sa, da), (sf, df), (sb, db) = tile3
    room = max(KA_t * 128 - len(sa), 0)
    x = min(room, len(sf))
    sA = np.concatenate([sa, sf[:x]])
    dA = np.concatenate([da, df[:x]])
    sB = np.concatenate([sf[x:] - OVER, sb - OVER])
    dB = np.concatenate([df[x:], db])
    return (sA, dA), (sB, dB)


def _slot_counts(graph_cores_list, TLOC):
    """Shared per-tile A/B slot counts (max across all datasets)."""
    KA = np.zeros(TLOC, np.int64)
    KB = np.zeros(TLOC, np.int64)
    for cores in graph_cores_list:
        for tiles in cores:
            for t in range(TLOC):
                (sa, _), _, _ = tiles[t]
                KA[t] = max(KA[t], (len(sa) + 127) // 128)
    KA = np.maximum(KA, 1)
    for cores in graph_cores_list:
        for tiles in cores:
            for t in range(TLOC):
                (sa, _), (sf, _), (sb, _) = tiles[t]
                x = min(max(KA[t] * 128 - len(sa), 0), len(sf))
                nB = len(sf) - x + len(sb)
                KB[t] = max(KB[t], (nB + 127) // 128)
    KB = np.maximum(KB, 1)
    return KA, KB


def _build_streams(tiles, KA, KB, TLOC, OVER):
    """Flatten one core's per-tile bucket lists into padded chunk streams."""
    idxA, dlA, idxB, dlB = [], [], [], []
    for t in range(TLOC):
        (sa, da), (sb, db) = _split_tile(tiles[t], KA[t], OVER)
        for (s_, d_, K, idx_l, dl_l) in (
            (sa, da, KA[t], idxA, dlA),
            (sb, db, KB[t], idxB, dlB),
        ):
            # ascending table rows within the tile bucket for DRAM locality
            o = np.argsort(s_, kind="stable")
            s_, d_ = s_[o], d_[o]
            n = s_.shape[0]
            slots = K * 128
            assert n <= slots, (n, slots)
            si = np.zeros(slots, np.int64)
            di = np.full(slots, -1.0, np.float32)
            si[:n] = s_
            di[:n] = d_.astype(np.float32)
            idx_l.append(si)
            dl_l.append(di)
    idxA = np.concatenate(idxA) if idxA else np.zeros(0, np.int64)
    idxB = np.concatenate(idxB) if idxB else np.zeros(0, np.int64)
    dlA = np.concatenate(dlA) if dlA else np.zeros(0, np.float32)
    dlB = np.concatenate(dlB) if dlB else np.zeros(0, np.float32)
    return (
        _pack_idx(idxA), _pack_cols(dlA, 128),
        _pack_idx(idxB), _pack_cols(dlB, 128),
    )


def _cols_from_vec(v_padded, TL):
    """[TL*128] -> [128, TL] per-tile columns."""
    return np.ascontiguousarray(v_padded.reshape(TL, 128).T)


# ---------------------------------------------------------------- builder

def _build_program(NP, SLICE, ABOUND, OVER, KA, KB, KA2, KB2, bounds,
                   n_cores_total, group_size):
    TLOC = SLICE // 128
    TFULL = NP // 128
    CA, CB = int(sum(KA)), int(sum(KB))
    CA2, CB2 = int(sum(KA2)), int(sum(KB2))
    LA, LB = CA * 128, CB * 128
    LA2, LB2 = CA2 * 128, CB2 * 128

    nc = bacc.Bacc("TRN2", target_bir_lowering=False, debug=False,
                   num_devices=n_cores_total, num_swdge_queues=NQ,
                   dynamic_dma_scratch_size=65536)

    xT = nc.dram_tensor("xT", [128, NP], BF16, kind="ExternalInput")
    rwrT = nc.dram_tensor("rwrT", [128, SLICE], BF16, kind="ExternalInput")
    xTloc = nc.dram_tensor("xTloc", [128, SLICE], BF16, kind="ExternalInput")
    idxA = nc.dram_tensor("idxA", [128, max(LA // 16, 1)], I16, kind="ExternalInput")
    idxB = nc.dram_tensor("idxB", [128, max(LB // 16, 1)], I16, kind="ExternalInput")
    dlA = nc.dram_tensor("dlA", [128, max(CA, 1)], BF16, kind="ExternalInput")
    dlB = nc.dram_tensor("dlB", [128, max(CB, 1)], BF16, kind="ExternalInput")
    idxA2 = nc.dram_tensor("idxA2", [128, max(LA2 // 16, 1)], I16, kind="ExternalInput")
    idxB2 = nc.dram_tensor("idxB2", [128, max(LB2 // 16, 1)], I16, kind="ExternalInput")
    dlA2 = nc.dram_tensor("dlA2", [128, max(CA2, 1)], BF16, kind="ExternalInput")
    dlB2 = nc.dram_tensor("dlB2", [128, max(CB2, 1)], BF16, kind="ExternalInput")
    dinv_full = nc.dram_tensor("dinv_full", [128, TFULL], F32, kind="ExternalInput")
    dinv_loc = nc.dram_tensor("dinv_loc", [128, TLOC], F32, kind="ExternalInput")
    dinvsq_loc = nc.dram_tensor("dinvsq_loc", [128, TLOC], F32, kind="ExternalInput")
    W1 = nc.dram_tensor("W1", [128, 128], BF16, kind="ExternalInput")
    W2 = nc.dram_tensor("W2", [128, 128], BF16, kind="ExternalInput")
    linW = nc.dram_tensor("linW", [128, 128], BF16, kind="ExternalInput")
    combWt = nc.dram_tensor("combWt", [128, 128], BF16, kind="ExternalInput")
    combWb = nc.dram_tensor("combWb", [128, 128], BF16, kind="ExternalInput")
    iota = nc.dram_tensor("iota", [128, 128], BF16, kind="ExternalInput")
    ident = nc.dram_tensor("ident", [128, 128], BF16, kind="ExternalInput")
    emd_out = nc.dram_tensor("emd_out", [SLICE, 128], F32, kind="ExternalOutput")

    groups = [
        list(range(g * group_size, (g + 1) * group_size))
        for g in range(n_cores_total // group_size)
    ]

    with tile.TileContext(nc) as tc:
        with tc.tile_pool(name="dram", bufs=1, space="DRAM") as dram, \
             tc.tile_pool(name="const", bufs=1) as cp, \
             tc.tile_pool(name="xload", bufs=3) as xp, \
             tc.tile_pool(name="blkA", bufs=3) as bap, \
             tc.tile_pool(name="blkB", bufs=2) as bbp, \
             tc.tile_pool(name="sA", bufs=2) as sap, \
             tc.tile_pool(name="sB", bufs=2) as sbp, \
             tc.tile_pool(name="work", bufs=3) as wp, \
             tc.tile_pool(name="norm", bufs=4) as npools, \
             tc.tile_pool(name="ps_agg", bufs=2, space="PSUM") as ps_agg, \
             tc.tile_pool(name="ps_aux", bufs=2, space="PSUM") as ps_aux, \
             tc.tile_pool(name="ps_tr", bufs=2, space="PSUM") as ps_tr:
            # PSUM budget (8 banks): ps_agg tag "agg" x2, ps_aux tag "mm" x2,
            # ps_tr tag "tr" x2 -> 6 bank-padded slots.

            h1t = dram.tile([NP, 128], BF16)
            h2slice = dram.tile([SLICE, 128], BF16)
            h2full = dram.tile([NP, 128], BF16)

            # ---- constants / streams in SBUF
            def cload(t_dram, shape, dt, tag):
                t_sb = cp.tile(shape, dt, tag=tag)
                nc.sync.dma_start(t_sb[:], t_dram[:, :])
                return t_sb

            idxA_t = cload(idxA, [128, max(LA // 16, 1)], I16, "idxA")
            idxB_t = cload(idxB, [128, max(LB // 16, 1)], I16, "idxB")
            dlA_t = cload(dlA, [128, max(CA, 1)], BF16, "dlA")
            dlB_t = cload(dlB, [128, max(CB, 1)], BF16, "dlB")
            idxA2_t = cload(idxA2, [128, max(LA2 // 16, 1)], I16, "idxA2")
            idxB2_t = cload(idxB2, [128, max(LB2 // 16, 1)], I16, "idxB2")
            dlA2_t = cload(dlA2, [128, max(CA2, 1)], BF16, "dlA2")
            dlB2_t = cload(dlB2, [128, max(CB2, 1)], BF16, "dlB2")
            dinvf_t = cload(dinv_full, [128, TFULL], F32, "dinvf")
            dinvl_t = cload(dinv_loc, [128, TLOC], F32, "dinvl")
            dinvsq_t = cload(dinvsq_loc, [128, TLOC], F32, "dinvsq")
            W1_t = cload(W1, [128, 128], BF16, "W1")
            W2_t = cload(W2, [128, 128], BF16, "W2")
            linW_t = cload(linW, [128, 128], BF16, "linW")
            combWt_t = cload(combWt, [128, 128], BF16, "combWt")
            combWb_t = cload(combWb, [128, 128], BF16, "combWb")
            iota_t = cload(iota, [128, 128], BF16, "iota")
            ident_t = cload(ident, [128, 128], BF16, "ident")

            Copy = mybir.ActivationFunctionType.Copy

            def l1norm_scale(src_ap, out_tile_ap):
                """out = src / max(sum|src|, EPS), per-partition rows."""
                s_sum = npools.tile([128, 1], F32, tag="nsum")
                nc.vector.reduce_sum(
                    s_sum[:], src_ap, axis=mybir.AxisListType.X,
                    apply_absolute_value=True)
                s_max = npools.tile([128, 1], F32, tag="nmax")
                nc.vector.tensor_scalar_max(s_max[:], s_sum[:], EPS)
                r = npools.tile([128, 1], F32, tag="nrec")
                nc.vector.reciprocal(r[:], s_max[:])
                nc.scalar.activation(out_tile_ap, src_ap, Copy, scale=r[:, 0:1])

            # ================= stage 1: h~1 full table (redundant per core)
            for tq in range(TFULL // 4):
                x4 = xp.tile([128, 4, 128], BF16, tag="x4")
                nc.sync.dma_start(x4[:], xT[:, tq * 512:(tq + 1) * 512]
                                  .rearrange("p (j f) -> p j f", j=4))
                o4 = xp.tile([128, 4, 128], BF16, tag="o4")
                for j in range(4):
                    t = tq * 4 + j
                    ps = ps_aux.tile([128, 128], F32, tag="mm")
                    nc.tensor.matmul(ps[:], lhsT=x4[:, j, :], rhs=W1_t[:],
                                     start=True, stop=True)
                    nc.scalar.activation(o4[:, j, :], ps[:], Copy,
                                         scale=dinvf_t[:, t:t + 1])
                nc.sync.dma_start(
                    h1t[tq * 512:(tq + 1) * 512, :]
                    .rearrange("(j p) f -> p j f", p=128),
                    o4[:])

            qctr = [0]

            def agg_pass(tableA_ap, tableB_ap, node_major, KAv, KBv,
                         idxA_sb, idxB_sb, dlA_sb, dlB_sb, CAv, CBv):
                """Runs the chunked aggregation over all local tiles.

                node_major=False: psum[dim, dst] += Hg^T-style (lhsT=Hg, rhs=S)
                node_major=True:  psum[dst, dim] += (lhsT=S, rhs=Hg)
                Yields (t, psum_tile) at each tile close.
                """
                qA = qB = 0
                blkA_t = blkB_t = sblkA_t = sblkB_t = None
                for t in range(TLOC):
                    ps = ps_agg.tile([128, 128], F32,
                                     tag="agg")
                    done = 0
                    for (K, stream_q, idx_t, dl_t, table_ap, pool, spool,
                         which) in (
                        (KAv[t], qA, idxA_sb, dlA_sb, tableA_ap, bap, sap, "A"),
                        (KBv[t], qB, idxB_sb, dlB_sb, tableB_ap, bbp, sbp, "B"),
                    ):
                        q = stream_q
                        CTOT = CAv if which == "A" else CBv
                        for i in range(K):
                            if q % GC == 0:
                                cb = min(GC, CTOT - q)
                                blk = pool.tile([128, GC, 128], BF16,
                                                tag="blk" + which)
                                nc.gpsimd.dma_gather(
                                    blk[:, :cb, :], table_ap,
                                    idx_t[:, q * 8:(q + cb) * 8],
                                    num_idxs=cb * 128, num_idxs_reg=cb * 128,
                                    elem_size=128, single_packet=SINGLE_PACKET,
                                    queue_num=GATHER_QS[qctr[0] % len(GATHER_QS)])
                                qctr[0] += 1
                                if which == "A":
                                    blkA_t = blk
                                else:
                                    blkB_t = blk
                            if q % SG == 0:
                                sb_ = min(SG, CTOT - q)
                                sblk = spool.tile([128, SG, 128], BF16,
                                                  tag="sblk" + which)
                                nc.vector.tensor_tensor(
                                    out=sblk[:, :sb_, :],
                                    in0=iota_t[:].unsqueeze(1)
                                        .broadcast_to([128, sb_, 128]),
                                    in1=dl_t[:, q:q + sb_].unsqueeze(2)
                                        .broadcast_to([128, sb_, 128]),
                                    op=mybir.AluOpType.is_equal)
                                if which == "A":
                                    sblkA_t = sblk
                                else:
                                    sblkB_t = sblk
                            blk = blkA_t if which == "A" else blkB_t
                            sblk = sblkA_t if which == "A" else sblkB_t
                            s_t = sblk[:, q % SG, :]
                            hg = blk[:, q % GC, :]
                            if node_major:
                                nc.tensor.matmul(ps[:], lhsT=s_t, rhs=hg,
                                                 start=(done == 0), stop=False)
                            else:
                                nc.tensor.matmul(ps[:], lhsT=hg, rhs=s_t,
                                                 start=(done == 0), stop=False)
                            q += 1
                            done += 1
                        if which == "A":
                            qA = q
                        else:
                            qB = q
                    yield t, ps

            # ================= stage 2: layer-1 agg (feature-major) -> h~2
            # slice, with the group AllGather pipelined chunk-by-chunk
            cc_next = 0  # index into bounds of the next chunk to exchange
            for t, ps in agg_pass(h1t[0:ABOUND, :], h1t[OVER:NP, :], False,
                                  KA, KB, idxA_t, idxB_t, dlA_t, dlB_t,
                                  CA, CB):
                # self-loop term: += W1^T @ (dinv*X_T)[:, own tile]
                xl = wp.tile([128, 128], BF16, tag="xl")
                nc.sync.dma_start(xl[:], xTloc[:, t * 128:(t + 1) * 128])
                nc.tensor.matmul(ps[:], lhsT=W1_t[:], rhs=xl[:],
                                 start=False, stop=True)
                # close: aggT[dim, dst] -> h~2 slice tile (node-major, scaled)
                aggT_sb = wp.tile([128, 128], BF16, tag="aggT")
                nc.scalar.activation(aggT_sb[:], ps[:], Copy)
                h2T_ps = ps_aux.tile([128, 128], F32, tag="mm")
                nc.tensor.matmul(h2T_ps[:], lhsT=W2_t[:], rhs=aggT_sb[:],
                                 start=True, stop=True)
                h2T_sb = wp.tile([128, 128], BF16, tag="h2Ts")
                nc.scalar.activation(h2T_sb[:], h2T_ps[:], Copy)
                h2_ps = ps_tr.tile([128, 128], BF16, tag="tr")
                nc.tensor.transpose(h2_ps[:], h2T_sb[:], ident_t[:])
                h2_sb = wp.tile([128, 128], BF16, tag="h2s")
                nc.scalar.activation(h2_sb[:], h2_ps[:], Copy,
                                     scale=dinvsq_t[:, t:t + 1])
                nc.sync.dma_start(h2slice[t * 128:(t + 1) * 128, :], h2_sb[:])
                # chunk complete -> exchange it while later tiles aggregate
                if t + 1 == bounds[cc_next + 1]:
                    rs, re = bounds[cc_next] * 128, bounds[cc_next + 1] * 128
                    nc.gpsimd.collective_compute(
                        "AllGather", mybir.AluOpType.bypass,
                        replica_groups=groups,
                        ins=[h2slice[rs:re, :].opt()],
                        outs=[h2full[group_size * rs:group_size * re, :].opt()])
                    cc_next += 1

            # ================= stage 4+5: layer-2 agg (node-major) + head
            for t, ps in agg_pass(h2full[0:ABOUND, :], h2full[OVER:NP, :], True,
                                  KA2, KB2, idxA2_t, idxB2_t, dlA2_t, dlB2_t,
                                  CA2, CB2):
                # self-loop term: += h~2[own tile] (identity matmul)
                h2s = wp.tile([128, 128], BF16, tag="h2self")
                nc.sync.dma_start(h2s[:], h2slice[t * 128:(t + 1) * 128, :])
                nc.tensor.matmul(ps[:], lhsT=ident_t[:], rhs=h2s[:],
                                 start=False, stop=True)
                # g = l1norm(dinv * agg2)
                g_pre = wp.tile([128, 128], F32, tag="gpre")
                nc.scalar.activation(g_pre[:], ps[:], Copy,
                                     scale=dinvl_t[:, t:t + 1])
                g_bf = wp.tile([128, 128], BF16, tag="gbf")
                l1norm_scale(g_pre[:], g_bf[:])
                gT_ps = ps_tr.tile([128, 128], BF16, tag="tr")
                nc.tensor.transpose(gT_ps[:], g_bf[:], ident_t[:])
                gT_sb = wp.tile([128, 128], BF16, tag="gT")
                nc.scalar.activation(gT_sb[:], gT_ps[:], Copy)

                # pos = l1norm(rwr @ linW)
                rw = wp.tile([128, 128], BF16, tag="rw")
                nc.sync.dma_start(rw[:], rwrT[:, t * 128:(t + 1) * 128])
                pos_ps = ps_aux.tile([128, 128], F32, tag="mm")
                nc.tensor.matmul(pos_ps[:], lhsT=rw[:], rhs=linW_t[:],
                                 start=True, stop=True)
                pos_bf = wp.tile([128, 128], BF16, tag="posbf")
                l1norm_scale(pos_ps[:], pos_bf[:])
                posT_ps = ps_tr.tile([128, 128], BF16, tag="tr")
                nc.tensor.transpose(posT_ps[:], pos_bf[:], ident_t[:])
                posT_sb = wp.tile([128, 128], BF16, tag="posT")
                nc.scalar.activation(posT_sb[:], posT_ps[:], Copy)

                # emd = l1norm(concat(pos, g) @ combW)
                emd_ps = ps_aux.tile([128, 128], F32, tag="mm")
                nc.tensor.matmul(emd_ps[:], lhsT=posT_sb[:], rhs=combWt_t[:],
                                 start=True, stop=False)
                nc.tensor.matmul(emd_ps[:], lhsT=gT_sb[:], rhs=combWb_t[:],
                                 start=False, stop=True)
                emd_f = wp.tile([128, 128], F32, tag="emdf")
                l1norm_scale(emd_ps[:], emd_f[:])
                nc.sync.dma_start(emd_out[t * 128:(t + 1) * 128, :], emd_f[:])

    nc.compile()
    return nc


# ---------------------------------------------------------------- kernel

def _run(inputs, N, E, n_cores_total=8, group_size=4):
    n_groups = n_cores_total // group_size
    assert n_groups == 2
    SLICE = ((N + group_size * 128 - 1) // (group_size * 128)) * 128
    NP = SLICE * group_size
    ABOUND = min(32768, NP)
    OVER = max(NP - 32768, 0)
    assert NP - OVER <= 32768
    TLOC = SLICE // 128
    TFULL = NP // 128
    assert TFULL % 4 == 0

    bf = ml_dtypes.bfloat16

    bounds = _chunk_bounds(TLOC)
    perm = _perm_from_bounds(bounds, SLICE, NP, group_size)

    graphs = []
    for g in range(2):
        ei = inputs["edge_index1" if g == 0 else "edge_index2"]
        dinv, dinvsq, cores = _prep_graph(ei, N, NP, SLICE, ABOUND, OVER,
                                          group_size)
        _, _, cores2 = _prep_graph(ei, N, NP, SLICE, ABOUND, OVER,
                                   group_size, src_map=perm)
        graphs.append((dinv, dinvsq, cores, cores2))

    KA, KB = _slot_counts([g[2] for g in graphs], TLOC)
    KA2, KB2 = _slot_counts([g[3] for g in graphs], TLOC)

    key = (NP, SLICE, ABOUND, OVER, tuple(KA), tuple(KB),
           tuple(KA2), tuple(KB2), tuple(bounds), n_cores_total, group_size)
    if key not in _prog_cache:
        _prog_cache[key] = _build_program(
            NP, SLICE, ABOUND, OVER, KA, KB, KA2, KB2, bounds,
            n_cores_total, group_size)
    nc = _prog_cache[key]

    iota_np = np.broadcast_to(
        np.arange(128, dtype=np.float32), (128, 128)).astype(bf)
    ident_np = np.eye(128, dtype=np.float32).astype(bf)
    W1_np = np.asarray(inputs["conv1_W"], np.float32).astype(bf)
    W2_np = np.asarray(inputs["conv2_W"], np.float32).astype(bf)
    linW_np = np.asarray(inputs["lin_W"], np.float32).astype(bf)
    combW = np.asarray(inputs["comb_W"], np.float32)
    combWt_np = combW[:128].astype(bf)
    combWb_np = combW[128:].astype(bf)

    in_maps = []
    for core in range(n_cores_total):
        g = core // group_size
        c = core % group_size
        dinv, dinvsq, cores, cores2 = graphs[g]
        x = np.asarray(inputs["x1" if g == 0 else "x2"], np.float32)
        rwr = np.asarray(inputs["rwr1_emd" if g == 0 else "rwr2_emd"],
                         np.float32)

        dinv_p = np.ones(NP, np.float32)
        dinv_p[:N] = dinv
        dinvsq_p = np.ones(NP, np.float32)
        dinvsq_p[:N] = dinvsq

        xT = np.zeros((128, NP), np.float32)
        xT[:, :N] = x.T
        rwrT = np.zeros((128, SLICE), np.float32)
        lo, hi = c * SLICE, min((c + 1) * SLICE, N)
        if hi > lo:
            rwrT[:, :hi - lo] = rwr[lo:hi].T

        iA, dA, iB, dB = _build_streams(cores[c], KA, KB, TLOC, OVER)
        iA2, dA2, iB2, dB2 = _build_streams(cores2[c], KA2, KB2, TLOC, OVER)

        sl = slice(c * SLICE, (c + 1) * SLICE)
        xTloc = xT[:, sl] * dinv_p[sl][None, :]
        in_maps.append({
            "xT": xT.astype(bf),
            "rwrT": rwrT.astype(bf),
            "xTloc": xTloc.astype(bf),
            "idxA": iA, "idxB": iB,
            "dlA": dA.astype(bf), "dlB": dB.astype(bf),
            "idxA2": iA2, "idxB2": iB2,
            "dlA2": dA2.astype(bf), "dlB2": dB2.astype(bf),
            "dinv_full": _cols_from_vec(dinv_p, TFULL),
            "dinv_loc": _cols_from_vec(dinv_p[sl], TLOC),
            "dinvsq_loc": _cols_from_vec(dinvsq_p[sl], TLOC),
            "W1": W1_np, "W2": W2_np, "linW": linW_np,
            "combWt": combWt_np, "combWb": combWb_np,
            "iota": iota_np, "ident": ident_np,
        })

    import os
    if os.environ.get("GCN_SIM"):
        from concourse.bass_interp import MultiCoreSim
        sim = MultiCoreSim(nc, num_cores=n_cores_total, trace=False,
                           require_finite=False, require_nnan=False)
        cores = list(sim.cores.values())
        for c, core_sim in enumerate(cores):
            for k, v in in_maps[c].items():
                core_sim.tensor(k)[:] = v
        sim.simulate(check_with_hw=False)

        class _R:
            results = [{"emd_out": np.array(core_sim.tensor("emd_out"))}
                       for core_sim in cores]
        res = _R()
    else:
        trace = bool(os.environ.get("GCN_TRACE"))
        if trace:
            import sys, types
            if "antenv.axon_hooks" not in sys.modules:
                mod = types.ModuleType("antenv.axon_hooks")
                mod._hook = None
                mod.set_axon_ntff_profile_hook = \
                    lambda h: setattr(mod, "_hook", h)
                mod.get_axon_ntff_profile_hook = lambda: mod._hook
                sys.modules["antenv.axon_hooks"] = mod
                from trn_agent_boot.trn_boot import _ntff_profile_via_ctypes
                mod.set_axon_ntff_profile_hook(
                    _ntff_profile_via_ctypes('/opt/axon/libaxon_pjrt.so'))
        res = run_bass_kernel_spmd(nc, in_maps,
                                   core_ids=list(range(n_cores_total)),
                                   trace=trace)
        if trace:
            print(f"HW exec time: {res.exec_time_ns} ns "
                  f"(mean {res.mean_exec_time_ns}, "
                  f"core {res.max_exec_time_core_id})")
            if res.instructions_and_trace:
                print("trace:", res.instructions_and_trace[1])

    outs = []
    for g in range(2):
        parts = [res.results[g * group_size + c]["emd_out"]
                 for c in range(group_size)]
        outs.append(np.concatenate(parts, axis=0)[:N])
    return outs[0], outs[1]


def kernel(rwr1_emd, rwr2_emd, x1, x2, edge_index1, edge_index2,
           lin_W, lin_b, conv1_W, conv1_b, conv2_W, conv2_b,
           comb_W, comb_b):
    for name, b in (("lin_b", lin_b), ("conv1_b", conv1_b),
                    ("conv2_b", conv2_b), ("comb_b", comb_b)):
        if np.any(np.asarray(b) != 0):
            raise NotImplementedError(f"nonzero bias {name} not supported")
    inputs = dict(rwr1_emd=rwr1_emd, rwr2_emd=rwr2_emd, x1=x1, x2=x2,
                  edge_index1=edge_index1, edge_index2=edge_index2,
                  lin_W=lin_W, conv1_W=conv1_W, conv2_W=conv2_W,
                  comb_W=comb_W)
    N = np.asarray(x1).shape[0]
    E = np.asarray(edge_index1).shape[1]
    return _run(inputs, N, E)
